# revision 26
# baseline (speedup 1.0000x reference)
"""Trainium2 Bass kernel for nn_CrossProduct (factorization-machine cross term).

out_b = 0.5 * [ ||x_b V||^2 - sum_n w_n x_bn^2 ],  w = rowsum(V^2)

Math restructuring (v2): ship xu = 4*(x*sqrt(w)) in bf16. Then
  term2_b = sum_n xu_bn^2 / 32          (plain square-sum, no weights)
  term1   = sum_k (xu_b @ V')^2,  V' = v/(4*u*sqrt(2))
so the PE never streams a weighted x^2 matmul:
  - 8 bf16 matmuls/tile: psumA[64,512] += V'[c].T @ xu[c]
  - squares xu^2 -> fp8e4m3 on ACT(2 chunks)/DVE(4)/GpSimd(2)
  - 4 fp8 DoubleRow matmuls/tile (2 chunks per stream, const -2^-5 lhsT)
    accumulate -term2 into psumO[1,512]
  - ACT: sq = Square(psumA) -> bf16; 1 bf16 matmul adds ones.T @ sq
  - ACT copies psumO -> out row; single 8KB DMA at the end.

Distribution: pure data-parallel over batch across 8 NeuronCores
(2048 rows/core); vparam-derived constants replicated.
"""

import math
from contextlib import ExitStack

import ml_dtypes
import numpy as np

import concourse.bass as bass
import concourse.bacc as bacc
import concourse.mybir as mybir
import concourse.tile as tile
from concourse.bass_utils import run_bass_kernel_spmd

BF16 = mybir.dt.bfloat16
F32 = mybir.dt.float32
F8 = mybir.dt.float8e4

N_CORES = 8
B, XD, KD = 16384, 1024, 64
BS = B // N_CORES  # 2048 batch rows per core
C = XD // 128      # 8 contraction chunks of 128
MT = 4             # b-tiles per core
BT = BS // MT      # 512 batch cols per tile

DR = mybir.MatmulPerfMode.DoubleRow
SQUARE = mybir.ActivationFunctionType.Square


def _body(ctx, tc, OUT, XU, VW, VF8):
    nc = tc.nc
    const = ctx.enter_context(tc.tile_pool(name="const", bufs=1))
    xpool = ctx.enter_context(tc.tile_pool(name="xp", bufs=4))
    qpool = ctx.enter_context(tc.tile_pool(name="qp", bufs=3))
    sqpool = ctx.enter_context(tc.tile_pool(name="sqp", bufs=2))
    opool = ctx.enter_context(tc.tile_pool(name="op", bufs=1))
    psA = ctx.enter_context(tc.tile_pool(name="psA", bufs=2, space="PSUM"))
    psO = ctx.enter_context(tc.tile_pool(name="psO", bufs=2, space="PSUM"))

    # vw columns: [0:64]=V', 64=1.0 (sq-reduce weights), 65=0.0 (ACT bias)
    # DMA plan: both HWDGE rings start immediately. sync: vw, tile0 in 4
    # chunk-pair slices (PE starts after the first 0.26MB), then tile2.
    # scalar: vf8, tile1, tile3 — issued BEFORE any dependent ACT work so
    # the ring is never blocked behind compute semaphores.
    vw = const.tile([128, C, 66], BF16)
    nc.sync.dma_start(vw[:], VW)
    vf8 = const.tile([128, 2, 32], F8)
    nc.scalar.dma_start(vf8[:], VF8)
    # All x on the sync ring, strictly FIFO in consumption order: two rings
    # halve each other's bandwidth on the shared DMA engines, so serial on
    # one ring feeds tile0 fastest and then stays ~1 tile ahead of the PE.
    xts = [xpool.tile([128, C, BT], BF16, name=f"xt{m}") for m in range(MT)]
    for h in range(4):
        nc.sync.dma_start(xts[0][:, 2 * h : 2 * h + 2], XU[:, 0, 2 * h : 2 * h + 2])
    nc.sync.dma_start(xts[1][:], XU[:, 1])
    nc.sync.dma_start(xts[2][:], XU[:, 2])
    nc.sync.dma_start(xts[3][:], XU[:, 3])
    outs = opool.tile([1, BS], F32)
    # one-time ACT touch of the vw DMA so later Square ops (which read the
    # bias column) carry only their data-dependency wait.
    actwarm = const.tile([128, 1], BF16)
    nc.scalar.copy(actwarm[:], vw[:, 0, 65:66])

    # PE p-state warmup: stream dummy matmuls on memset scratch while the
    # first x tile is still in flight, so real matmuls start at full clock.
    wpool = ctx.enter_context(tc.tile_pool(name="wp", bufs=1))
    psW = ctx.enter_context(tc.tile_pool(name="psW", bufs=1, space="PSUM"))
    scratch = wpool.tile([128, BT], BF16)
    nc.gpsimd.memset(scratch[:], 0)
    pw = psW.tile([64, BT], F32)
    for _ in range(4):
        nc.tensor.matmul(
            pw[:],
            scratch[:, 0:64],
            scratch[:],
            start=True,
            stop=True,
            tile_position=(0, 0),
        )

    # squares -> fp8, split across engines (chunk pairs align with the
    # DoubleRow rhs groups so each DR matmul has a single producer).
    # ACT emission is software-pipelined: the next tile's first xq square is
    # enqueued BEFORE this tile's PSUM-dependent sq square, so ACT is never
    # blocked waiting on the PE when it could be squaring already-landed x.
    xqs = [qpool.tile([128, C, BT], F8, name=f"xq{m}") for m in range(MT)]
    nc.scalar.activation(xqs[0][:, 0:2], xts[0][:, 0:2], SQUARE, bias=vw[:, 0, 65:66])
    nc.scalar.activation(xqs[0][:, 2:4], xts[0][:, 2:4], SQUARE, bias=vw[:, 0, 65:66])

    pas = [None] * MT

    def emit_tail(j):
        # term2 DR matmuls + sq reduce for tile j, deferred one tile so the
        # fp8 square producers get a full extra tile of slack (no PE waits)
        po = psO.tile([32, BT], F32, name="po")
        for t in range(4):
            nc.tensor.matmul(
                po[:],
                vf8[:],
                xqs[j][:, 2 * t : 2 * t + 2],
                start=(t == 0),
                stop=False,
                perf_mode=DR,
            )
        sq = sqpool.tile([64, BT], BF16, name="sq")
        nc.scalar.activation(sq[:], pas[j][:], SQUARE, bias=vw[0:64, 0, 65:66])
        nc.tensor.matmul(
            po[0:1, :],
            vw[0:64, 0, 64:65],
            sq[:],
            start=False,
            stop=True,
        )
        # out-row copy on DVE, then ship immediately on GpSimd's DMA queue
        # (~25ns issue, doesn't block the sync x feed).
        nc.vector.tensor_scalar_add(outs[:, j * BT : (j + 1) * BT], po[0:1, :], 0.0)
        nc.gpsimd.dma_start(OUT[:, j * BT : (j + 1) * BT], outs[:, j * BT : (j + 1) * BT])

    for m in range(MT):
        xt, xq = xts[m], xqs[m]
        nc.vector.tensor_mul(xq[:, 4:6], xt[:, 4:6], xt[:, 4:6])
        nc.gpsimd.tensor_mul(xq[:, 6:8], xt[:, 6:8], xt[:, 6:8])

        pa = psA.tile([64, BT], F32, name="pa")
        pas[m] = pa
        for c in range(C):
            nc.tensor.matmul(
                pa[:],
                vw[:, c, 0:64],
                xt[:, c],
                start=(c == 0),
                stop=(c == C - 1),
                tile_position=(0, 0),
            )
        if m + 1 < MT:
            nc.scalar.activation(
                xqs[m + 1][:, 0:2], xts[m + 1][:, 0:2], SQUARE, bias=vw[:, 0, 65:66]
            )
            nc.scalar.activation(
                xqs[m + 1][:, 2:4], xts[m + 1][:, 2:4], SQUARE, bias=vw[:, 0, 65:66]
            )
        if m > 0:
            emit_tail(m - 1)
    emit_tail(MT - 1)


_NC_CACHE = None


def build_nc():
    global _NC_CACHE
    if _NC_CACHE is not None:
        return _NC_CACHE
    nc = bacc.Bacc("TRN2", target_bir_lowering=False, debug=False)
    XU = nc.dram_tensor("XU", [128, MT, C, BT], BF16, kind="ExternalInput").ap()
    VW = nc.dram_tensor("VW", [128, C, 66], BF16, kind="ExternalInput").ap()
    VF8 = nc.dram_tensor("VF8", [128, 2, 32], F8, kind="ExternalInput").ap()
    OUT = nc.dram_tensor("OUT", [1, BS], F32, kind="ExternalOutput").ap()
    with tile.TileContext(nc) as tc:
        with ExitStack() as ctx:
            _body(ctx, tc, OUT, XU, VW, VF8)
    nc.compile()
    _NC_CACHE = nc
    return nc


def make_in_maps(x, vparam):
    bf = ml_dtypes.bfloat16
    f8 = ml_dtypes.float8_e4m3
    x = np.ascontiguousarray(x, dtype=np.float32)
    v = np.ascontiguousarray(vparam, dtype=np.float64)

    w = (v**2).sum(axis=1)                      # (1024,)
    u = np.sqrt(w)
    vs = (v / (u[:, None] * 4.0 * math.sqrt(2.0))).astype(np.float32).astype(bf)
    xu = (x.astype(np.float64) * (4.0 * u)[None, :]).astype(np.float32)

    VWh = np.empty((128, C, 66), dtype=bf)
    VWh[:, :, 0:64] = np.asarray(vs).reshape(C, 128, KD).transpose(1, 0, 2)
    VWh[:, :, 64] = bf(1.0)
    VWh[:, :, 65] = bf(0.0)

    VF8h = np.zeros((128, 2, 32), dtype=f8)
    VF8h[:, :, 0] = f8(-(2.0**-5))

    in_maps = []
    for i in range(N_CORES):
        xs = xu[i * BS : (i + 1) * BS]               # (2048, 1024)
        xt = np.ascontiguousarray(xs.T)              # (1024, 2048) [k, b]
        A = xt.reshape(C, 128, MT, BT).transpose(1, 2, 0, 3)
        XUh = np.ascontiguousarray(A).astype(bf)
        in_maps.append({"XU": XUh, "VW": VWh, "VF8": VF8h})
    return in_maps


LAST_RESULTS = None  # stashed BassKernelResults (for test harness profiling)
TRACE = False


def kernel(x, vparam):
    global LAST_RESULTS
    nc = build_nc()
    in_maps = make_in_maps(x, vparam)
    res = run_bass_kernel_spmd(nc, in_maps, list(range(N_CORES)), trace=TRACE)
    LAST_RESULTS = res
    out = np.concatenate(
        [res.results[i]["OUT"].reshape(BS, 1) for i in range(N_CORES)], axis=0
    )
    return out.astype(np.float32)
